# revision 14
# baseline (speedup 1.0000x reference)
"""Causal multi-head attention (B=2, S=2048, D=1024, H=16) on 8 trn2
NeuronCores.

Sharding: batch*heads across cores — core c handles batch c//4 and heads
4*(c%4) .. 4*(c%4)+3 (a 256-wide slice of the q/k/v feature dim).  W_proj is
tensor-parallel split along the head dim, so each core emits a full-shape
[S, D] partial projection output; the host sums the 4 partials per batch.

Per-core layout strategy (everything fp32, matmuls in float32r for full PE
rate):
  - host feeds x[b].T so the contraction dim (d) lands on partitions
  - qT/kT computed in transposed [dh, s] layout (2 head-pairs of 128)
  - v computed in natural [s, dh] layout with a ones column appended, so the
    attention AV matmul also produces the softmax denominator row for free
  - scores computed transposed ([sk, sq] strips) with causal raggedness —
    only the valid lower-triangular blocks are computed; the diagonal block
    is masked by accumulating one bf16 matmul (strict-upper -1000 against
    identity) before Exp
  - softmax needs no max-subtraction (scores ~ N(0,1); exp cannot overflow)
  - attnT [65, 2048] PSUM accumulation; row 64 = denominators; normalize via
    DVE reciprocal + DMA partition-broadcast + tensor_mul
  - projection consumes attnT directly as lhsT; partial out -> DRAM
"""

import os

import numpy as np

# cache compiled executables (incl. the wrapped NEFF) across processes
os.environ.setdefault("JAX_COMPILATION_CACHE_DIR", "/tmp/jax_comp_cache")
os.environ.setdefault("JAX_PERSISTENT_CACHE_MIN_ENTRY_SIZE_BYTES", "0")
os.environ.setdefault("JAX_PERSISTENT_CACHE_MIN_COMPILE_TIME_SECS", "0")

S = 2048
D = 1024
DH = 64
P = 128
NT = S // P   # 16 sequence tiles
DC = D // P   # 8 contraction chunks
MASK_C = 1000.0
N_CORES = 8

_CACHE = {}


def _pieces_for_strip(j):
    """Column pieces [a,b) for scoresT strip j (valid sq >= j*128);
    512-aligned after the first ragged piece so PSUM banks are respected."""
    a = j * P
    out = []
    first_end = min(S, ((a // 512) + 1) * 512)
    if a < first_end:
        out.append((a, first_end))
    b = first_end
    while b < S:
        out.append((b, b + 512))
        b += 512
    return out


def _build_bass():
    import concourse.bass as bass
    import concourse.tile as tile
    from concourse import mybir

    f32 = mybir.dt.float32
    f32r = mybir.dt.float32r
    bf16 = mybir.dt.bfloat16
    EXP = mybir.ActivationFunctionType.Exp

    nc = bass.Bass("TRN2")

    xT_d = nc.dram_tensor("xT", [D, S], f32r, kind="ExternalInput")
    wq_d = nc.dram_tensor("wq_t", [D, 256], f32r, kind="ExternalInput")
    wk_d = nc.dram_tensor("wk_t", [D, 256], f32r, kind="ExternalInput")
    wv_d = nc.dram_tensor("wv_t", [D, 256], f32r, kind="ExternalInput")
    wp_d = nc.dram_tensor("wp_t", [256, D], f32r, kind="ExternalInput")
    mask_d = nc.dram_tensor("mask_lhsT", [P, P], bf16, kind="ExternalInput")
    ident_d = nc.dram_tensor("ident", [P, P], bf16, kind="ExternalInput")
    out_d = nc.dram_tensor("out", [S, D], f32, kind="ExternalOutput")

    with tile.TileContext(nc) as tc:
        with tc.tile_pool(name="persist", bufs=1) as persist:
            qT = [persist.tile([P, S], f32r, name=f"qT{p}", tag=f"qT{p}")
                  for p in range(2)]
            kT = [persist.tile([P, S], f32r, name=f"kT{p}", tag=f"kT{p}")
                  for p in range(2)]
            # per sk-tile: 4 heads x [64 v-cols | 64 ones-cols].  The ones
            # half makes the AV matmul emit 64 replicated denominator rows,
            # so normalization needs no cross-partition broadcast.
            v4e = [persist.tile([P, 4, P], f32r, name=f"v4e{t}", tag=f"v4e{t}")
                   for t in range(NT)]
            wp_sb = [persist.tile([P, D], f32r, name=f"wp{p}", tag=f"wp{p}")
                     for p in range(2)]
            attnT = [persist.tile([P, S], f32r, name=f"attnT{p}", tag=f"attnT{p}")
                     for p in range(2)]
            mask_sb = persist.tile([P, P], bf16, name="mask_sb", tag="mask_sb")
            ident_sb = persist.tile([P, P], bf16, name="ident_sb", tag="ident_sb")

            nc.sync.dma_start(out=mask_sb[:], in_=mask_d[:])
            nc.sync.dma_start(out=ident_sb[:], in_=ident_d[:])
            for p in range(2):
                nc.sync.dma_start(out=wp_sb[p][:], in_=wp_d[p * P:(p + 1) * P, :])

            # ---------------- phase 1: q/k/v projections ----------------
            with tc.tile_pool(name="xw", bufs=1) as xw, \
                 tc.tile_pool(name="pk", bufs=2, space="PSUM") as pk:
                xTt = [xw.tile([P, S], f32r, name=f"xTt{c}", tag=f"xTt{c}")
                       for c in range(DC)]
                wq_sb = [xw.tile([P, 256], f32r, name=f"wq{c}", tag=f"wq{c}")
                         for c in range(DC)]
                wk_sb = [xw.tile([P, 256], f32r, name=f"wk{c}", tag=f"wk{c}")
                         for c in range(DC)]
                wv_sb = [xw.tile([P, 256], f32r, name=f"wv{c}", tag=f"wv{c}")
                         for c in range(DC)]
                for c in range(DC):
                    nc.sync.dma_start(out=wq_sb[c][:], in_=wq_d[c * P:(c + 1) * P, :])
                    nc.sync.dma_start(out=wk_sb[c][:], in_=wk_d[c * P:(c + 1) * P, :])
                    nc.sync.dma_start(out=wv_sb[c][:], in_=wv_d[c * P:(c + 1) * P, :])
                    nc.sync.dma_start(out=xTt[c][:], in_=xT_d[c * P:(c + 1) * P, :])

                # qT / kT: [128, 2048] per head-pair, transposed layout
                for dst, wsb in ((qT, wq_sb), (kT, wk_sb)):
                    for p in range(2):
                        for n in range(4):
                            ps = pk.tile([P, 512], f32, name="ps_qk", tag="ps_qk")
                            for c in range(DC):
                                nc.tensor.matmul(
                                    ps[:],
                                    lhsT=wsb[c][:, p * P:(p + 1) * P],
                                    rhs=xTt[c][:, n * 512:(n + 1) * 512],
                                    start=(c == 0), stop=(c == DC - 1),
                                )
                            nc.vector.tensor_copy(
                                dst[p][:, n * 512:(n + 1) * 512], ps[:])

                # v natural [s, 256] -> interleaved [128, 4, 128]: per head
                # 64 v-cols then 64 ones-cols (memset to f32r is not a legal
                # ISA instruction, so ones go through an f32 scratch + copy)
                ones_sb = xw.tile([P, 256], f32, name="ones_sb", tag="ones_sb")
                nc.vector.memset(ones_sb[:], 1.0)
                for t in range(NT):
                    ps = pk.tile([P, 256], f32, name="ps_v", tag="ps_v")
                    for c in range(DC):
                        nc.tensor.matmul(
                            ps[:],
                            lhsT=xTt[c][:, t * P:(t + 1) * P],
                            rhs=wv_sb[c][:],
                            start=(c == 0), stop=(c == DC - 1),
                        )
                    nc.vector.tensor_copy(
                        v4e[t][:, :, 0:64],
                        ps.rearrange("p (h d) -> p h d", h=4))
                    nc.vector.tensor_copy(
                        v4e[t][:, :, 64:P],
                        ones_sb.rearrange("p (h d) -> p h d", h=4))

            # ---------------- phase 2: attention per local head ----------------
            with tc.tile_pool(name="att", bufs=2) as att, \
                 tc.tile_pool(name="po", bufs=4) as po:
                with tc.tile_pool(name="ps_s", bufs=2, space="PSUM") as ps_s, \
                     tc.tile_pool(name="ps_a", bufs=1, space="PSUM") as ps_a:

                    for k in range(4):
                        pair, half = k // 2, k % 2
                        qh = qT[pair][half * 64:(half + 1) * 64, :]
                        kh = kT[pair][half * 64:(half + 1) * 64, :]

                        pa = ps_a.tile([P, S], f32, name="pa", tag="pa")
                        for j in range(NT):
                            w0 = j * P
                            pieces = _pieces_for_strip(j)
                            # scoresT strip [w0, S) split into <=1024-wide PSUM
                            # segments (2 banks each); segment bases must be
                            # 512-aligned globally so pieces stay in-bank
                            if w0 < 1024:
                                segs = [(0, 1024), (1024, S)]
                            else:
                                segs = [(1024, S)]
                            seg_tiles = {}
                            for (sa, sb) in segs:
                                st = ps_s.tile([P, 1024], f32, name="strip",
                                               tag="strip")
                                seg_tiles[sa] = (st, sa, sb)

                            def seg_ap(a, b):
                                for (st, sa, sb) in seg_tiles.values():
                                    if a >= sa and b <= sb:
                                        return st[:, a - sa:b - sa]
                                raise AssertionError((a, b))

                            for (a, b) in pieces:
                                diag_here = a <= w0 < b
                                nc.tensor.matmul(
                                    seg_ap(a, b),
                                    lhsT=kh[:, w0:w0 + P],
                                    rhs=qh[:, a:b],
                                    start=True, stop=not diag_here,
                                    skip_group_check=True,
                                )
                            # additive causal mask on the diagonal block:
                            # strip[p, w0+q] += mask_lhsT[q, p] (= -C for q < p)
                            nc.tensor.matmul(
                                seg_ap(w0, w0 + P),
                                lhsT=mask_sb[:],
                                rhs=ident_sb[:],
                                start=False, stop=True,
                                skip_group_check=True,
                            )
                            et = att.tile([P, S], f32r, name="et", tag="et",
                                          bufs=3)
                            for (st, sa, sb) in seg_tiles.values():
                                lo = max(sa, w0)
                                nc.scalar.activation(
                                    out=et[:, lo:sb],
                                    in_=st[:, lo - sa:sb - sa],
                                    func=EXP)
                            for (a, b) in pieces:
                                nc.tensor.matmul(
                                    pa[:, a:b],
                                    lhsT=v4e[j][:, k, :],
                                    rhs=et[:, a:b],
                                    start=(j == 0),
                                    stop=(j == min(NT - 1, (b - 1) // P)),
                                    skip_group_check=True,
                                )

                        recip = att.tile([64, S], f32, name="recip", tag="recip")
                        nc.vector.reciprocal(recip[:], pa[64:P, :])
                        nc.vector.tensor_mul(
                            attnT[pair][half * 64:(half + 1) * 64, :],
                            pa[0:64, :],
                            recip[:],
                        )

                # ---------------- phase 3: output projection ----------------
                with tc.tile_pool(name="ps_o", bufs=4, space="PSUM") as ps_o:
                    for t in range(NT):
                        for oc in range(2):
                            pso = ps_o.tile([P, 512], f32, name="pso", tag="pso")
                            for p in range(2):
                                nc.tensor.matmul(
                                    pso[:],
                                    lhsT=attnT[p][:, t * P:(t + 1) * P],
                                    rhs=wp_sb[p][:, oc * 512:(oc + 1) * 512],
                                    start=(p == 0), stop=(p == 1),
                                )
                            ot = po.tile([P, 512], f32, name="ot", tag="ot")
                            nc.vector.tensor_copy(ot[:], pso[:])
                            nc.sync.dma_start(
                                out=out_d[t * P:(t + 1) * P,
                                          oc * 512:(oc + 1) * 512],
                                in_=ot[:])

    _fix_matmul_waits(nc)
    return nc


def _fix_matmul_waits(nc):
    """The TRN2 ISA events struct holds exactly ONE sync-wait per
    instruction and walrus codegen refuses instructions carrying more
    ("Too many sync wait commands").  Tile emits multi-wait instructions,
    so legalize: hoist excess waits onto single-wait NoOps inserted right
    before the instruction on the same engine — engine FIFO order
    preserves the synchronization semantics."""
    import bass_rust
    import concourse.mybir as mybir

    n = 0
    for bb in nc.main_func.blocks:
        insts = bb.instructions
        i = 0
        while i < len(insts):
            ins = insts[i]
            si = getattr(ins, "sync_info", None)
            if si is not None and len(si.on_wait) >= 2:
                for w in si.on_wait[:-1]:
                    nop = mybir.InstNoOp(name=f"I-xwait-{n}", ins=[], outs=[])
                    nop.engine = ins.engine
                    nop.sync_info = bass_rust.SyncInfo(
                        on_wait=[w], on_update=[])
                    insts.insert(i, nop)
                    n += 1
                    i += 1
                ins.sync_info = bass_rust.SyncInfo(
                    on_wait=[si.on_wait[-1]], on_update=si.on_update)
            i += 1
    return n


def get_nc():
    if "nc" not in _CACHE:
        _CACHE["nc"] = _build_bass()
    return _CACHE["nc"]


def make_in_maps(x, W_q, W_k, W_v, W_proj):
    import ml_dtypes

    x = np.asarray(x, np.float32)
    W_q = np.asarray(W_q, np.float32)
    W_k = np.asarray(W_k, np.float32)
    W_v = np.asarray(W_v, np.float32)
    W_proj = np.asarray(W_proj, np.float32)

    mask = np.triu(np.full((P, P), -MASK_C, np.float32), k=1)
    mask = mask.astype(ml_dtypes.bfloat16)
    ident = np.eye(P, dtype=ml_dtypes.bfloat16)

    xTs = [np.ascontiguousarray(x[b].T) for b in range(2)]
    in_maps = []
    for core in range(N_CORES):
        b = core // 4
        g = core % 4
        rs = slice(g * 256, (g + 1) * 256)
        in_maps.append({
            "xT": xTs[b],
            "wq_t": np.ascontiguousarray(W_q[rs].T) / 8.0,
            "wk_t": np.ascontiguousarray(W_k[rs].T),
            "wv_t": np.ascontiguousarray(W_v[rs].T),
            "wp_t": np.ascontiguousarray(W_proj[:, rs].T),
            "mask_lhsT": mask,
            "ident": ident,
        })
    return in_maps


def kernel(x, W_q, W_k, W_v, W_proj, _results_hook=None):
    from concourse.bass_utils import run_bass_kernel_spmd

    nc = get_nc()
    in_maps = make_in_maps(x, W_q, W_k, W_v, W_proj)
    res = run_bass_kernel_spmd(nc, in_maps, core_ids=list(range(N_CORES)))
    if _results_hook is not None:
        _results_hook(res)
    out = np.zeros((2, S, D), np.float32)
    for core in range(N_CORES):
        out[core // 4] += res.results[core]["out"]
    return out


if __name__ == "__main__":
    nc = get_nc()
    print("built ok; instructions:",
          sum(len(bb.instructions) for bb in nc.main_func.blocks))


# revision 21
# speedup vs baseline: 309.0515x; 309.0515x over previous
"""Causal multi-head attention (B=2, S=2048, D=1024, H=16) on 8 trn2
NeuronCores.

Sharding (per the head-parallel hint): core c handles batch c//4 and heads
4*(c%4) .. 4*(c%4)+3 (a 256-wide slice of the q/k/v feature dim).  W_proj is
tensor-parallel split along the head dim, so each core emits a full-shape
[S, D] partial projection output; the host sums the 4 partials per batch.

Per-core layout strategy (fp32 data, float32r matmuls for full PE rate):
  - host feeds x[b].T so the contraction dim (d) lands on partitions
  - qT/kT computed in transposed [dh, s] layout (2 head-pairs of 128)
  - v computed in natural [s, dh] layout, interleaved per head as
    [64 v-cols | 64 ones-cols], so the AV matmul also emits 64 replicated
    softmax-denominator rows -- normalization needs no cross-partition
    broadcast
  - scoresT strips [sk, sq] computed quarter-major with causal raggedness;
    both heads of a pair share one [128, 1024] PSUM strip (disjoint PE row
    groups run their kq matmuls concurrently; one strided Exp covers both)
  - diagonal blocks masked by accumulating one bf16 matmul (strict-upper
    -1000 against identity) before Exp; softmax needs no max subtraction
    (scores ~ N(0,1), exp cannot overflow; exp(masked) flushes to 0)
  - projection consumes the normalized attnT directly as lhsT

The TRN2 ISA holds one sync-wait per instruction; Tile emits more, so
excess waits are hoisted onto same-engine NoOps after scheduling.
"""

import os

import numpy as np

# cache compiled executables (incl. the wrapped NEFF) across processes
os.environ.setdefault("JAX_COMPILATION_CACHE_DIR", "/tmp/jax_comp_cache")
os.environ.setdefault("JAX_PERSISTENT_CACHE_MIN_ENTRY_SIZE_BYTES", "0")
os.environ.setdefault("JAX_PERSISTENT_CACHE_MIN_COMPILE_TIME_SECS", "0")

S = 2048
D = 1024
DH = 64
P = 128
NT = S // P   # 16 sequence tiles
DC = D // P   # 8 contraction chunks
MASK_C = 1000.0
N_CORES = 8

_CACHE = {}


def _build_bass():
    import concourse.bass as bass
    import concourse.tile as tile
    from concourse import mybir

    f32 = mybir.dt.float32
    f32r = mybir.dt.float32r
    bf16 = mybir.dt.bfloat16
    EXP = mybir.ActivationFunctionType.Exp

    nc = bass.Bass("TRN2")

    xT_d = nc.dram_tensor("xT", [D, S], f32r, kind="ExternalInput")
    wq_d = nc.dram_tensor("wq_t", [D, 256], f32r, kind="ExternalInput")
    wk_d = nc.dram_tensor("wk_t", [D, 256], f32r, kind="ExternalInput")
    wv_d = nc.dram_tensor("wv_t", [D, 256], f32r, kind="ExternalInput")
    wp_d = nc.dram_tensor("wp_t", [256, D], f32r, kind="ExternalInput")
    mask_d = nc.dram_tensor("mask_lhsT", [P, P], bf16, kind="ExternalInput")
    ident_d = nc.dram_tensor("ident", [P, P], bf16, kind="ExternalInput")
    out_d = nc.dram_tensor("out", [S, D], f32, kind="ExternalOutput")

    with tile.TileContext(nc) as tc:
        with tc.tile_pool(name="persist", bufs=1) as persist:
            qT = [persist.tile([P, S], f32r, name=f"qT{p}", tag=f"qT{p}")
                  for p in range(2)]
            kT = [persist.tile([P, S], f32r, name=f"kT{p}", tag=f"kT{p}")
                  for p in range(2)]
            # per sk-tile: 4 heads x [64 v-cols | 64 ones-cols]; the ones
            # half makes the AV matmul emit 64 replicated denominator rows
            v4e = [persist.tile([P, 4, P], f32r, name=f"v4e{t}", tag=f"v4e{t}")
                   for t in range(NT)]
            wp_sb = [persist.tile([P, D], f32r, name=f"wp{p}", tag=f"wp{p}")
                     for p in range(2)]
            attnT = [persist.tile([P, S], f32r, name=f"attnT{p}", tag=f"attnT{p}")
                     for p in range(2)]
            mask_sb = persist.tile([P, P], bf16, name="mask_sb", tag="mask_sb")
            ident_sb = persist.tile([P, P], bf16, name="ident_sb", tag="ident_sb")

            nc.sync.dma_start(out=mask_sb[:], in_=mask_d[:])
            nc.sync.dma_start(out=ident_sb[:], in_=ident_d[:])
            for p in range(2):
                nc.sync.dma_start(out=wp_sb[p][:], in_=wp_d[p * P:(p + 1) * P, :])

            # ---------------- phase 1: q/k/v projections ----------------
            with tc.tile_pool(name="xw", bufs=1) as xw, \
                 tc.tile_pool(name="pk", bufs=2, space="PSUM") as pk:
                xTt = [xw.tile([P, S], f32r, name=f"xTt{c}", tag=f"xTt{c}")
                       for c in range(DC)]
                wq_sb = [xw.tile([P, 256], f32r, name=f"wq{c}", tag=f"wq{c}")
                         for c in range(DC)]
                wk_sb = [xw.tile([P, 256], f32r, name=f"wk{c}", tag=f"wk{c}")
                         for c in range(DC)]
                wv_sb = [xw.tile([P, 256], f32r, name=f"wv{c}", tag=f"wv{c}")
                         for c in range(DC)]
                for c in range(DC):
                    nc.sync.dma_start(out=wq_sb[c][:], in_=wq_d[c * P:(c + 1) * P, :])
                    nc.sync.dma_start(out=wk_sb[c][:], in_=wk_d[c * P:(c + 1) * P, :])
                    nc.sync.dma_start(out=wv_sb[c][:], in_=wv_d[c * P:(c + 1) * P, :])
                    nc.sync.dma_start(out=xTt[c][:], in_=xT_d[c * P:(c + 1) * P, :])

                # qT / kT: [128, 2048] per head-pair, transposed layout.
                # Order: pair-0 q/k, then v, then pair-1 q/k -- pair-0
                # attention only waits on the first part of this phase.
                def qk_pair(p):
                    for dst, wsb in ((qT, wq_sb), (kT, wk_sb)):
                        for n in range(4):
                            ps = pk.tile([P, 512], f32, name="ps_qk",
                                         tag="ps_qk")
                            for c in range(DC):
                                nc.tensor.matmul(
                                    ps[:],
                                    lhsT=wsb[c][:, p * P:(p + 1) * P],
                                    rhs=xTt[c][:, n * 512:(n + 1) * 512],
                                    start=(c == 0), stop=(c == DC - 1),
                                )
                            nc.vector.tensor_copy(
                                dst[p][:, n * 512:(n + 1) * 512], ps[:])

                qk_pair(0)

                ones_sb = xw.tile([P, 256], f32, name="ones_sb", tag="ones_sb")
                nc.vector.memset(ones_sb[:], 1.0)
                for t in range(NT):
                    ps = pk.tile([P, 256], f32, name="ps_v", tag="ps_v")
                    for c in range(DC):
                        nc.tensor.matmul(
                            ps[:],
                            lhsT=xTt[c][:, t * P:(t + 1) * P],
                            rhs=wv_sb[c][:],
                            start=(c == 0), stop=(c == DC - 1),
                        )
                    nc.vector.tensor_copy(
                        v4e[t][:, :, 0:64],
                        ps.rearrange("p (h d) -> p h d", h=4))
                    nc.vector.tensor_copy(
                        v4e[t][:, :, 64:P],
                        ones_sb.rearrange("p (h d) -> p h d", h=4))

                qk_pair(1)

            # ---------------- phase 2: attention, head-pair packed ----------
            with tc.tile_pool(name="att", bufs=2) as att, \
                 tc.tile_pool(name="po", bufs=4) as po:
                with tc.tile_pool(name="ps_s", bufs=2, space="PSUM") as ps_s, \
                     tc.tile_pool(name="ps_a", bufs=4, space="PSUM") as ps_a:

                    for pr in range(2):
                        qh = [qT[pr][h * 64:(h + 1) * 64, :] for h in range(2)]
                        kh = [kT[pr][h * 64:(h + 1) * 64, :] for h in range(2)]

                        for qc in range(4):          # 512-col sq quarter
                            c0 = qc * 512
                            pa = [ps_a.tile([P, 512], f32, name=f"pa{h}",
                                            tag=f"pa{h}", bufs=2)
                                  for h in range(2)]
                            jmax = min(4 * qc + 3, NT - 1)
                            for j in range(jmax + 1):
                                w0 = j * P
                                lo = max(w0, c0)     # first valid col
                                w = c0 + 512 - lo
                                strip = ps_s.tile([P, 1024], f32,
                                                  name="strip", tag="strip")
                                for h in range(2):
                                    nc.tensor.matmul(
                                        strip[:, h * 512 + lo - c0:
                                              h * 512 + lo - c0 + w],
                                        lhsT=kh[h][:, w0:w0 + P],
                                        rhs=qh[h][:, lo:lo + w],
                                        start=True, stop=(j // 4 != qc),
                                        skip_group_check=True,
                                    )
                                if j // 4 == qc:
                                    # diagonal block: additive causal mask
                                    # strip[p, q] += mask_lhsT[q, p]
                                    for h in range(2):
                                        nc.tensor.matmul(
                                            strip[:, h * 512 + w0 - c0:
                                                  h * 512 + w0 - c0 + P],
                                            lhsT=mask_sb[:],
                                            rhs=ident_sb[:],
                                            start=False, stop=True,
                                            skip_group_check=True,
                                        )
                                et = att.tile([P, 1024], f32r, name="et",
                                              tag="et", bufs=4)
                                sv = strip.rearrange("p (h q) -> p h q", h=2)
                                ev = et.rearrange("p (h q) -> p h q", h=2)
                                nc.scalar.activation(
                                    out=ev[:, :, lo - c0:lo - c0 + w],
                                    in_=sv[:, :, lo - c0:lo - c0 + w],
                                    func=EXP)
                                for h in range(2):
                                    k_loc = 2 * pr + h
                                    nc.tensor.matmul(
                                        pa[h][:, lo - c0:lo - c0 + w],
                                        lhsT=v4e[j][:, k_loc, :],
                                        rhs=et[:, h * 512 + lo - c0:
                                               h * 512 + lo - c0 + w],
                                        start=(j == 0), stop=(j == jmax),
                                        skip_group_check=True,
                                    )
                            for h in range(2):
                                recip = att.tile([64, 512], f32, name="recip",
                                                 tag="recip", bufs=4)
                                nc.vector.reciprocal(recip[:], pa[h][64:P, :])
                                nc.vector.tensor_mul(
                                    attnT[pr][h * 64:(h + 1) * 64,
                                              c0:c0 + 512],
                                    pa[h][0:64, :],
                                    recip[:],
                                )

                # ---------------- phase 3: output projection ----------------
                with tc.tile_pool(name="ps_o", bufs=4, space="PSUM") as ps_o:
                    for t in range(NT):
                        for oc in range(2):
                            pso = ps_o.tile([P, 512], f32, name="pso", tag="pso")
                            for p in range(2):
                                nc.tensor.matmul(
                                    pso[:],
                                    lhsT=attnT[p][:, t * P:(t + 1) * P],
                                    rhs=wp_sb[p][:, oc * 512:(oc + 1) * 512],
                                    start=(p == 0), stop=(p == 1),
                                )
                            ot = po.tile([P, 512], f32, name="ot", tag="ot")
                            nc.scalar.copy(ot[:], pso[:])
                            nc.sync.dma_start(
                                out=out_d[t * P:(t + 1) * P,
                                          oc * 512:(oc + 1) * 512],
                                in_=ot[:])

    return nc


def _fix_matmul_waits(nc):
    """The TRN2 ISA events struct holds exactly ONE sync-wait per
    instruction and walrus codegen refuses instructions carrying more
    ("Too many sync wait commands").  Tile emits multi-wait instructions,
    so legalize: hoist excess waits onto single-wait NoOps inserted right
    before the instruction on the same engine -- engine FIFO order
    preserves the synchronization semantics."""
    import bass_rust
    import concourse.mybir as mybir

    n = 0
    for bb in nc.main_func.blocks:
        insts = bb.instructions
        i = 0
        while i < len(insts):
            ins = insts[i]
            si = getattr(ins, "sync_info", None)
            if si is not None and len(si.on_wait) >= 2:
                for w in si.on_wait[:-1]:
                    nop = mybir.InstNoOp(name=f"I-xwait-{n}", ins=[], outs=[])
                    nop.engine = ins.engine
                    nop.sync_info = bass_rust.SyncInfo(
                        on_wait=[w], on_update=[])
                    insts.insert(i, nop)
                    n += 1
                    i += 1
                ins.sync_info = bass_rust.SyncInfo(
                    on_wait=[si.on_wait[-1]], on_update=si.on_update)
            i += 1
    return n


def get_nc(legalize=True):
    key = ("nc", legalize)
    if key not in _CACHE:
        nc = _build_bass()
        if legalize:
            _fix_matmul_waits(nc)
        _CACHE[key] = nc
    return _CACHE[key]


def make_in_maps(x, W_q, W_k, W_v, W_proj):
    import ml_dtypes

    x = np.asarray(x, np.float32)
    W_q = np.asarray(W_q, np.float32)
    W_k = np.asarray(W_k, np.float32)
    W_v = np.asarray(W_v, np.float32)
    W_proj = np.asarray(W_proj, np.float32)

    mask = np.triu(np.full((P, P), -MASK_C, np.float32), k=1)
    mask = mask.astype(ml_dtypes.bfloat16)
    ident = np.eye(P, dtype=ml_dtypes.bfloat16)

    xTs = [np.ascontiguousarray(x[b].T) for b in range(2)]
    in_maps = []
    for core in range(N_CORES):
        b = core // 4
        g = core % 4
        rs = slice(g * 256, (g + 1) * 256)
        in_maps.append({
            "xT": xTs[b],
            "wq_t": np.ascontiguousarray(W_q[rs].T) / 8.0,
            "wk_t": np.ascontiguousarray(W_k[rs].T),
            "wv_t": np.ascontiguousarray(W_v[rs].T),
            "wp_t": np.ascontiguousarray(W_proj[:, rs].T),
            "mask_lhsT": mask,
            "ident": ident,
        })
    return in_maps


def kernel(x, W_q, W_k, W_v, W_proj, _results_hook=None):
    from concourse.bass_utils import run_bass_kernel_spmd

    nc = get_nc()
    in_maps = make_in_maps(x, W_q, W_k, W_v, W_proj)
    res = run_bass_kernel_spmd(nc, in_maps, core_ids=list(range(N_CORES)))
    if _results_hook is not None:
        _results_hook(res)
    out = np.zeros((2, S, D), np.float32)
    for core in range(N_CORES):
        out[core // 4] += res.results[core]["out"]
    return out


if __name__ == "__main__":
    nc = get_nc()
    print("built ok; instructions:",
          sum(len(bb.instructions) for bb in nc.main_func.blocks))
